# revision 2
# baseline (speedup 1.0000x reference)
"""PointPillarScatter (intersweep, 3 bins) Trainium2 Bass kernel.

Problem: for each of 3 bins, scatter 64000 pillar rows [64 feats] into a
[B=4, C=64, NY=496, NX=432] fp32 canvas at (b, :, y, x); empty cells zero.

Strategy (8 NeuronCores, SPMD), v2:
  - Shard the 12 (bin, b) canvases into 48 quarter-canvases of 124 y-rows;
    6 per core, processed as 3 pairs (A, B).  One output "group" = one
    y-row of a pair: [128 ch (A:0-64, B:64-128), 432 cells].
  - Everything on-device is fp16 (tolerance is 2e-2 rel; fp16 round-trip
    is ~2.4e-4).  The output canvas is stored fp16 and upcast on host;
    this halves the dominant out-DMA bytes vs fp32.
  - Per group one fp16 matmul places the pillars:
      acc[128, 432] = lhsT[128, 128].T @ onehot[128, 432]
    K rows hold this group's pillars, variably packed (A's then B's,
    n <= 128; features sit in their quarter's 64-col block, other block
    zero).  onehot[k, c] = (x[k] == c) built by one DVE/Pool
    tensor_scalar is_equal against an fp16 iota row (0..431 exact in
    fp16); unused rows have x = -1 -> all-zero mask rows.
  - lhsT is loaded pre-expanded from DRAM ([128, 372, 128] fp16 per
    core, 12.2 MB) - expanding compact features on-chip costs far more
    engine time than the extra DMA bytes.
  - PSUM -> SBUF copy converts fp32 -> fp16, split between ACT and DVE;
    masks split between DVE and Pool (ratios tuned from traces).
  - One [128 x 53568 B] contiguous DMA per 62-group chunk writes out
    (sync ring); lhst chunk loads ride the scalar ring.
"""

import numpy as np

import concourse.bass as bass
import concourse.tile as tile
from concourse import bacc, mybir
from concourse.bass_utils import run_bass_kernel_spmd

# Problem geometry (hardcoded; kernel.py must be self-contained).
B = 4
C = 64
NX = 432
NY = 496
NBINS = 3
NCORES = 8

NQ = NBINS * B * 4  # 48 quarter-canvases
YQ = NY // 4  # 124 y-rows per quarter
QPC = NQ // NCORES  # 6 quarters per core
PAIRS = QPC // 2  # 3 pairs per core
GPP = YQ  # groups (y-rows) per pair
G = PAIRS * GPP  # 372 groups per core
CH = 62  # groups per staging chunk
NCHUNKS = G // CH  # 6 chunks per core
KMAX = 128  # pillar slots per group (max observed ~90)

# Engine split patterns (index mod cycle -> alternate engine).
MASK_DVE_OF = (1, 2)  # 1 of 2 masks on DVE, rest Pool
COPY_ACT_OF = (3, 4)  # 3 of 4 copies on ACT, rest DVE

_cache = {}


def _build():
    nc = bacc.Bacc(trn_type="TRN2")
    f16 = mybir.dt.float16
    f32 = mybir.dt.float32
    lhst_d = nc.dram_tensor("lhst", [KMAX, G, KMAX], f16, kind="ExternalInput")
    iota_d = nc.dram_tensor("iotat", [KMAX, NX], f16, kind="ExternalInput")
    relc_d = nc.dram_tensor("relc", [KMAX, G], f32, kind="ExternalInput")
    out_d = nc.dram_tensor("out", [PAIRS, 2, KMAX, CH, NX], f16,
                           kind="ExternalOutput")

    with tile.TileContext(nc) as tc:
        with (
            tc.tile_pool(name="const", bufs=1) as constp,
            tc.tile_pool(name="lhstp", bufs=2) as lhstp,
            tc.tile_pool(name="maskp", bufs=8) as maskp,
            tc.tile_pool(name="stage", bufs=2) as stagep,
            tc.tile_pool(name="psum", bufs=6, space=bass.MemorySpace.PSUM) as psump,
        ):
            iota = constp.tile([KMAX, NX], f16, name="iota")
            relc = constp.tile([KMAX, G], f32, name="relc")
            nc.scalar.dma_start(out=iota[:], in_=iota_d[:])
            nc.scalar.dma_start(out=relc[:], in_=relc_d[:])
            for ci in range(NCHUNKS):
                g0 = ci * CH
                lt = lhstp.tile([KMAX, CH, KMAX], f16, name="lt")
                nc.scalar.dma_start(out=lt[:], in_=lhst_d[:, g0:g0 + CH, :])
                st = stagep.tile([KMAX, CH, NX], f16, name="st")
                for j in range(CH):
                    g = g0 + j
                    mask = maskp.tile([KMAX, NX], f16, name="mask")
                    meng = (nc.vector if (j % MASK_DVE_OF[1]) < MASK_DVE_OF[0]
                            else nc.gpsimd)
                    meng.tensor_scalar(
                        out=mask[:],
                        in0=iota[:],
                        scalar1=relc[:, g:g + 1],
                        scalar2=None,
                        op0=mybir.AluOpType.is_equal,
                    )
                    acc = psump.tile([KMAX, NX], f32, name="acc")
                    nc.tensor.matmul(acc[:], lt[:, j, :], mask[:],
                                     start=True, stop=True)
                    if (j % COPY_ACT_OF[1]) < COPY_ACT_OF[0]:
                        nc.scalar.copy(st[:, j, :], acc[:])
                    else:
                        nc.vector.tensor_copy(out=st[:, j, :], in_=acc[:])
                pair, half = divmod(ci, NCHUNKS // PAIRS)
                nc.sync.dma_start(out=out_d[pair, half], in_=st[:])
    nc.compile()
    return nc


def _pack(inputs):
    """Build per-core lhst/relc arrays (vectorized)."""
    lhst = np.zeros((NCORES, KMAX, G, KMAX), np.float16)
    relc = np.full((NCORES, KMAX, G), -1.0, np.float32)
    iota = np.broadcast_to(np.arange(NX, dtype=np.float16), (KMAX, NX))

    cores = []
    Gs = []
    halves = []
    xs = []
    feats_list = []
    for bin_i in range(NBINS):
        feats = np.asarray(inputs[f"pillar_features_bin_{bin_i}"]).astype(np.float16)
        coords = np.asarray(inputs[f"voxel_coords_bin_{bin_i}"])
        cb = coords[:, 0].astype(np.int64)
        cy = coords[:, 2].astype(np.int64)
        cx = coords[:, 3].astype(np.int64)
        yq = cy // YQ
        q = bin_i * 16 + cb * 4 + yq  # global quarter id
        core, jj = np.divmod(q, QPC)
        pair, half = np.divmod(jj, 2)
        Garr = pair * GPP + (cy - yq * YQ)
        cores.append(core)
        Gs.append(Garr)
        halves.append(half)
        xs.append(cx)
        feats_list.append(feats)

    core = np.concatenate(cores)
    Garr = np.concatenate(Gs)
    half = np.concatenate(halves)
    x = np.concatenate(xs)
    feats = np.concatenate(feats_list, axis=0)

    # stable order by (core, G, half); slot k = rank within (core, G)
    order = np.lexsort((half, Garr, core))
    core, Garr, half, x = core[order], Garr[order], half[order], x[order]
    feats = feats[order]
    key = (core * G + Garr)
    first = np.r_[True, key[1:] != key[:-1]]
    start = np.maximum.accumulate(np.where(first, np.arange(len(key)), 0))
    k = np.arange(len(key)) - start
    if k.max() >= KMAX:
        raise OverflowError(int(k.max()))

    ha = half == 0
    lhst[core[ha], k[ha], Garr[ha], 0:C] = feats[ha]
    hb = ~ha
    lhst[core[hb], k[hb], Garr[hb], C:2 * C] = feats[hb]
    relc[core, k, Garr] = x

    return [{"lhst": lhst[c], "iotat": iota, "relc": relc[c]}
            for c in range(NCORES)]


def _run(inputs, trace=False):
    if "nc" not in _cache:
        _cache["nc"] = _build()
    nc = _cache["nc"]
    in_maps = _pack(inputs)
    res = run_bass_kernel_spmd(nc, in_maps, core_ids=list(range(NCORES)),
                               trace=trace)
    outs = [np.zeros((B, C, NY, NX), np.float32) for _ in range(NBINS)]
    for q in range(NQ):
        bin_i, rem = divmod(q, 16)
        b, yq = divmod(rem, 4)
        core, jj = divmod(q, QPC)
        pair, half = divmod(jj, 2)
        blk = res.results[core]["out"][pair]  # [2, 128, CH, NX] f16
        arr = blk.transpose(1, 0, 2, 3).reshape(KMAX, YQ, NX)
        outs[bin_i][b, :, YQ * yq:YQ * (yq + 1), :] = arr[half * C:(half + 1) * C]
    return tuple(outs), res


def kernel(**inputs):
    out, _ = _run(inputs)
    return out


def kernel_traced(**inputs):
    """Like kernel() but also returns BassKernelResults (for test.py)."""
    return _run(inputs, trace=True)


# revision 4
# speedup vs baseline: 1.3950x; 1.3950x over previous
"""PointPillarScatter (intersweep, 3 bins) Trainium2 Bass kernel.

Problem: for each of 3 bins, scatter 64000 pillar rows [64 feats] into a
[B=4, C=64, NY=496, NX=432] fp32 canvas at (b, :, y, x); empty cells zero.

Strategy (8 NeuronCores, SPMD), v2:
  - Shard the 12 (bin, b) canvases into 48 quarter-canvases of 124 y-rows;
    6 per core, processed as 3 pairs (A, B).  One output "group" = one
    y-row of a pair: [128 ch (A:0-64, B:64-128), 432 cells].
  - Everything on-device is fp16 (tolerance is 2e-2 rel; fp16 round-trip
    is ~2.4e-4).  The output canvas is stored fp16 and upcast on host;
    this halves the dominant out-DMA bytes vs fp32.
  - Per group one fp16 matmul places the pillars:
      acc[128, 432] = lhsT[128, 128].T @ onehot[128, 432]
    K rows hold this group's pillars, variably packed (A's then B's,
    n <= 128; features sit in their quarter's 64-col block, other block
    zero).  onehot[k, c] = (x[k] == c) built by one DVE tensor_scalar
    is_equal against an fp16 iota row (0..431 exact in fp16); unused
    rows have x = -1 -> all-zero mask rows.  GpSimd is NEVER used: its
    tensor ops run ~20x slower and stall concurrent DVE work (shared
    SBUF ports).
  - lhsT is loaded pre-expanded from DRAM ([128, 372, 128] fp16 per
    core, 12.2 MB) - expanding compact features on-chip costs far more
    engine time than the extra DMA bytes.
  - Matmuls write fp16 PSUM; copies to SBUF staging are 4-groups-wide
    strided reads over 4 PSUM banks (amortize the ~260 ns fixed ACT
    overhead), split ACT / DVE by a tuned ratio.
  - One [128 x 53568 B] contiguous DMA per 62-group chunk writes out
    (sync ring); lhst chunk loads ride the scalar ring.
"""

import numpy as np

import concourse.bass as bass
import concourse.tile as tile
from concourse import bacc, mybir
from concourse.bass_utils import run_bass_kernel_spmd

# Problem geometry (hardcoded; kernel.py must be self-contained).
B = 4
C = 64
NX = 432
NY = 496
NBINS = 3
NCORES = 8

NQ = NBINS * B * 4  # 48 quarter-canvases
YQ = NY // 4  # 124 y-rows per quarter
QPC = NQ // NCORES  # 6 quarters per core
PAIRS = QPC // 2  # 3 pairs per core
GPP = YQ  # groups (y-rows) per pair
G = PAIRS * GPP  # 372 groups per core
CH = 62  # groups per staging chunk
NCHUNKS = G // CH  # 6 chunks per core
KMAX = 128  # pillar slots per group (max observed ~90)

PSUM_F16 = False  # matmul output must be fp32 (bass asserts)
DVE_COPY_QUADS = (7,)  # which 4-group copies per chunk go to DVE (rest ACT)

_cache = {}


def _build():
    nc = bacc.Bacc(trn_type="TRN2")
    f16 = mybir.dt.float16
    f32 = mybir.dt.float32
    acc_dt = f16 if PSUM_F16 else f32
    lhst_d = nc.dram_tensor("lhst", [KMAX, G, KMAX], f16, kind="ExternalInput")
    iota_d = nc.dram_tensor("iotat", [KMAX, NX], f16, kind="ExternalInput")
    relc_d = nc.dram_tensor("relc", [KMAX, G], f32, kind="ExternalInput")
    out_d = nc.dram_tensor("out", [PAIRS, 2, KMAX, CH, NX], f16,
                           kind="ExternalOutput")

    nquads = (CH + 3) // 4  # 16 (last quad holds 2 groups)

    with tile.TileContext(nc) as tc:
        with (
            tc.tile_pool(name="const", bufs=1) as constp,
            tc.tile_pool(name="lhstp", bufs=2) as lhstp,
            tc.tile_pool(name="maskp", bufs=8) as maskp,
            tc.tile_pool(name="stage", bufs=2) as stagep,
            tc.tile_pool(name="psum", bufs=4 if PSUM_F16 else 2,
                         space=bass.MemorySpace.PSUM) as psump,
        ):
            iota = constp.tile([KMAX, NX], f16, name="iota")
            relc = constp.tile([KMAX, G], f32, name="relc")
            nc.scalar.dma_start(out=iota[:], in_=iota_d[:])
            nc.scalar.dma_start(out=relc[:], in_=relc_d[:])
            for ci in range(NCHUNKS):
                g0 = ci * CH
                lt = lhstp.tile([KMAX, CH, KMAX], f16, name="lt")
                nc.scalar.dma_start(out=lt[:], in_=lhst_d[:, g0:g0 + CH, :])
                st = stagep.tile([KMAX, CH, NX], f16, name="st")
                for qi in range(nquads):
                    w = min(4, CH - 4 * qi)
                    acc = psump.tile([KMAX, 4, 512], acc_dt, name="acc")
                    for l in range(w):
                        j = 4 * qi + l
                        mask = maskp.tile([KMAX, NX], f16, name="mask")
                        nc.vector.tensor_scalar(
                            out=mask[:],
                            in0=iota[:],
                            scalar1=relc[:, g0 + j:g0 + j + 1],
                            scalar2=None,
                            op0=mybir.AluOpType.is_equal,
                        )
                        nc.tensor.matmul(acc[:, l, 0:NX], lt[:, j, :], mask[:],
                                         start=True, stop=True)
                    dst = st[:, 4 * qi:4 * qi + w, :]
                    src = acc[:, 0:w, 0:NX]
                    if qi in DVE_COPY_QUADS:
                        nc.vector.tensor_copy(out=dst, in_=src)
                    else:
                        nc.scalar.copy(dst, src)
                pair, half = divmod(ci, NCHUNKS // PAIRS)
                nc.sync.dma_start(out=out_d[pair, half], in_=st[:])
    nc.compile()
    return nc


def _pack(inputs):
    """Build per-core lhst/relc arrays (vectorized)."""
    lhst = np.zeros((NCORES, KMAX, G, KMAX), np.float16)
    relc = np.full((NCORES, KMAX, G), -1.0, np.float32)
    iota = np.broadcast_to(np.arange(NX, dtype=np.float16), (KMAX, NX))

    cores = []
    Gs = []
    halves = []
    xs = []
    feats_list = []
    for bin_i in range(NBINS):
        feats = np.asarray(inputs[f"pillar_features_bin_{bin_i}"]).astype(np.float16)
        coords = np.asarray(inputs[f"voxel_coords_bin_{bin_i}"])
        cb = coords[:, 0].astype(np.int64)
        cy = coords[:, 2].astype(np.int64)
        cx = coords[:, 3].astype(np.int64)
        yq = cy // YQ
        q = bin_i * 16 + cb * 4 + yq  # global quarter id
        core, jj = np.divmod(q, QPC)
        pair, half = np.divmod(jj, 2)
        Garr = pair * GPP + (cy - yq * YQ)
        cores.append(core)
        Gs.append(Garr)
        halves.append(half)
        xs.append(cx)
        feats_list.append(feats)

    core = np.concatenate(cores)
    Garr = np.concatenate(Gs)
    half = np.concatenate(halves)
    x = np.concatenate(xs)
    feats = np.concatenate(feats_list, axis=0)

    # stable order by (core, G, half); slot k = rank within (core, G)
    order = np.lexsort((half, Garr, core))
    core, Garr, half, x = core[order], Garr[order], half[order], x[order]
    feats = feats[order]
    key = (core * G + Garr)
    first = np.r_[True, key[1:] != key[:-1]]
    start = np.maximum.accumulate(np.where(first, np.arange(len(key)), 0))
    k = np.arange(len(key)) - start
    if k.max() >= KMAX:
        raise OverflowError(int(k.max()))

    ha = half == 0
    lhst[core[ha], k[ha], Garr[ha], 0:C] = feats[ha]
    hb = ~ha
    lhst[core[hb], k[hb], Garr[hb], C:2 * C] = feats[hb]
    relc[core, k, Garr] = x

    return [{"lhst": lhst[c], "iotat": iota, "relc": relc[c]}
            for c in range(NCORES)]


def _run(inputs, trace=False):
    if "nc" not in _cache:
        _cache["nc"] = _build()
    nc = _cache["nc"]
    in_maps = _pack(inputs)
    res = run_bass_kernel_spmd(nc, in_maps, core_ids=list(range(NCORES)),
                               trace=trace)
    outs = [np.zeros((B, C, NY, NX), np.float32) for _ in range(NBINS)]
    for q in range(NQ):
        bin_i, rem = divmod(q, 16)
        b, yq = divmod(rem, 4)
        core, jj = divmod(q, QPC)
        pair, half = divmod(jj, 2)
        blk = res.results[core]["out"][pair]  # [2, 128, CH, NX] f16
        arr = blk.transpose(1, 0, 2, 3).reshape(KMAX, YQ, NX)
        outs[bin_i][b, :, YQ * yq:YQ * (yq + 1), :] = arr[half * C:(half + 1) * C]
    return tuple(outs), res


def kernel(**inputs):
    out, _ = _run(inputs)
    return out


def kernel_traced(**inputs):
    """Like kernel() but also returns BassKernelResults (for test.py)."""
    return _run(inputs, trace=True)


# revision 7
# speedup vs baseline: 1.4824x; 1.0626x over previous
"""PointPillarScatter (intersweep, 3 bins) Trainium2 Bass kernel.

Problem: for each of 3 bins, scatter 64000 pillar rows [64 feats] into a
[B=4, C=64, NY=496, NX=432] fp32 canvas at (b, :, y, x); empty cells zero.

Strategy (8 NeuronCores, SPMD), v2:
  - Shard the 12 (bin, b) canvases into 48 quarter-canvases of 124 y-rows;
    6 per core, processed as 3 pairs (A, B).  One output "group" = one
    y-row of a pair: [128 ch (A:0-64, B:64-128), 432 cells].
  - Everything on-device is fp16 (tolerance is 2e-2 rel; fp16 round-trip
    is ~2.4e-4).  The output canvas is stored fp16 and upcast on host;
    this halves the dominant out-DMA bytes vs fp32.
  - Per group one fp16 matmul places the pillars:
      acc[128, 432] = lhsT[128, 128].T @ onehot[128, 432]
    K rows hold this group's pillars, variably packed (A's then B's,
    n <= 128; features sit in their quarter's 64-col block, other block
    zero).  onehot[k, c] = (x[k] == c) built by one DVE tensor_scalar
    is_equal against an fp16 iota row (0..431 exact in fp16); unused
    rows have x = -1 -> all-zero mask rows.  GpSimd is NEVER used: its
    tensor ops run ~20x slower and stall concurrent DVE work (shared
    SBUF ports).
  - lhsT is loaded pre-expanded from DRAM ([128, 372, 128] fp16 per
    core, 12.2 MB) - expanding compact features on-chip costs far more
    engine time than the extra DMA bytes.
  - Matmuls write fp16 PSUM; copies to SBUF staging are 4-groups-wide
    strided reads over 4 PSUM banks (amortize the ~260 ns fixed ACT
    overhead), split ACT / DVE by a tuned ratio.
  - One [128 x 53568 B] contiguous DMA per 62-group chunk writes out
    (sync ring); lhst chunk loads ride the scalar ring.
"""

import numpy as np

import concourse.bass as bass
import concourse.tile as tile
from concourse import bacc, mybir
from concourse.bass_utils import run_bass_kernel_spmd

# Problem geometry (hardcoded; kernel.py must be self-contained).
B = 4
C = 64
NX = 432
NY = 496
NBINS = 3
NCORES = 8

NQ = NBINS * B * 4  # 48 quarter-canvases
YQ = NY // 4  # 124 y-rows per quarter
QPC = NQ // NCORES  # 6 quarters per core
PAIRS = QPC // 2  # 3 pairs per core
GPP = YQ  # groups (y-rows) per pair
G = PAIRS * GPP  # 372 groups per core
CH = 31  # groups per staging chunk
NCHUNKS = G // CH  # 12 chunks per core
LTCH = 62  # groups per lhst tile (2 chunks)
KMAX = 128  # pillar slots per group (max observed ~90)

PSUM_F16 = False  # matmul output must be fp32 (bass asserts)
DVE_COPY_QUADS = (4,)  # which 4-group copies go to DVE (rest ACT), odd chunks

_cache = {}


def _build():
    nc = bacc.Bacc(trn_type="TRN2")
    f16 = mybir.dt.float16
    f32 = mybir.dt.float32
    acc_dt = f16 if PSUM_F16 else f32
    lhst_d = nc.dram_tensor("lhst", [KMAX, G, KMAX], f16, kind="ExternalInput")
    iota_d = nc.dram_tensor("iotat", [KMAX, NX], f16, kind="ExternalInput")
    relc_d = nc.dram_tensor("relc", [KMAX, G], f32, kind="ExternalInput")
    out_d = nc.dram_tensor("out", [PAIRS, NCHUNKS // PAIRS, KMAX, CH, NX], f16,
                           kind="ExternalOutput")

    nquads = (CH + 3) // 4  # 16 (last quad holds 2 groups)

    with tile.TileContext(nc) as tc:
        with (
            tc.tile_pool(name="const", bufs=1) as constp,
            tc.tile_pool(name="lhstp", bufs=2) as lhstp,
            tc.tile_pool(name="maskp", bufs=8) as maskp,
            tc.tile_pool(name="stage", bufs=2) as stagep,
            tc.tile_pool(name="psum", bufs=4 if PSUM_F16 else 2,
                         space=bass.MemorySpace.PSUM) as psump,
        ):
            iota = constp.tile([KMAX, NX], f16, name="iota")
            relc = constp.tile([KMAX, G], f32, name="relc")
            nc.scalar.dma_start(out=iota[:], in_=iota_d[:])
            nc.scalar.dma_start(out=relc[:], in_=relc_d[:])
            # all lhst upfront (6 tiles, 2 chunks each) so the in-loads don't
            # contend with out-DMAs for the rest of the run
            lts = [constp.tile([KMAX, LTCH, KMAX], f16, name=f"lt{i}")
                   for i in range(G // LTCH)]
            for i, lt in enumerate(lts):
                nc.scalar.dma_start(out=lt[:],
                                    in_=lhst_d[:, i * LTCH:(i + 1) * LTCH, :])
            for ci in range(NCHUNKS):
                g0 = ci * CH
                lt = lts[ci // 2]
                lj0 = (ci % 2) * CH
                st = stagep.tile([KMAX, CH, NX], f16, name="st")
                for qi in range(nquads):
                    w = min(4, CH - 4 * qi)
                    acc = psump.tile([KMAX, 4, 512], acc_dt, name="acc")
                    for l in range(w):
                        j = 4 * qi + l
                        mask = maskp.tile([KMAX, NX], f16, name="mask")
                        nc.vector.tensor_scalar(
                            out=mask[:],
                            in0=iota[:],
                            scalar1=relc[:, g0 + j:g0 + j + 1],
                            scalar2=None,
                            op0=mybir.AluOpType.is_equal,
                        )
                        nc.tensor.matmul(acc[:, l, 0:NX], lt[:, lj0 + j, :],
                                         mask[:], start=True, stop=True)
                    dst = st[:, 4 * qi:4 * qi + w, :]
                    src = acc[:, 0:w, 0:NX]
                    if qi in DVE_COPY_QUADS and ci % 2 == 1:
                        nc.vector.tensor_copy(out=dst, in_=src)
                    else:
                        nc.scalar.copy(dst, src)
                pair, half = divmod(ci, NCHUNKS // PAIRS)
                nc.sync.dma_start(out=out_d[pair, half], in_=st[:])
    nc.compile()
    return nc


def _pack(inputs):
    """Build per-core lhst/relc arrays (vectorized)."""
    lhst = np.zeros((NCORES, KMAX, G, KMAX), np.float16)
    relc = np.full((NCORES, KMAX, G), -1.0, np.float32)
    iota = np.broadcast_to(np.arange(NX, dtype=np.float16), (KMAX, NX))

    cores = []
    Gs = []
    halves = []
    xs = []
    feats_list = []
    for bin_i in range(NBINS):
        feats = np.asarray(inputs[f"pillar_features_bin_{bin_i}"]).astype(np.float16)
        coords = np.asarray(inputs[f"voxel_coords_bin_{bin_i}"])
        cb = coords[:, 0].astype(np.int64)
        cy = coords[:, 2].astype(np.int64)
        cx = coords[:, 3].astype(np.int64)
        yq = cy // YQ
        q = bin_i * 16 + cb * 4 + yq  # global quarter id
        core, jj = np.divmod(q, QPC)
        pair, half = np.divmod(jj, 2)
        Garr = pair * GPP + (cy - yq * YQ)
        cores.append(core)
        Gs.append(Garr)
        halves.append(half)
        xs.append(cx)
        feats_list.append(feats)

    core = np.concatenate(cores)
    Garr = np.concatenate(Gs)
    half = np.concatenate(halves)
    x = np.concatenate(xs)
    feats = np.concatenate(feats_list, axis=0)

    # stable order by (core, G, half); slot k = rank within (core, G)
    order = np.lexsort((half, Garr, core))
    core, Garr, half, x = core[order], Garr[order], half[order], x[order]
    feats = feats[order]
    key = (core * G + Garr)
    first = np.r_[True, key[1:] != key[:-1]]
    start = np.maximum.accumulate(np.where(first, np.arange(len(key)), 0))
    k = np.arange(len(key)) - start
    if k.max() >= KMAX:
        raise OverflowError(int(k.max()))

    ha = half == 0
    lhst[core[ha], k[ha], Garr[ha], 0:C] = feats[ha]
    hb = ~ha
    lhst[core[hb], k[hb], Garr[hb], C:2 * C] = feats[hb]
    relc[core, k, Garr] = x

    return [{"lhst": lhst[c], "iotat": iota, "relc": relc[c]}
            for c in range(NCORES)]


def _run(inputs, trace=False):
    if "nc" not in _cache:
        _cache["nc"] = _build()
    nc = _cache["nc"]
    in_maps = _pack(inputs)
    res = run_bass_kernel_spmd(nc, in_maps, core_ids=list(range(NCORES)),
                               trace=trace)
    outs = [np.zeros((B, C, NY, NX), np.float32) for _ in range(NBINS)]
    for q in range(NQ):
        bin_i, rem = divmod(q, 16)
        b, yq = divmod(rem, 4)
        core, jj = divmod(q, QPC)
        pair, half = divmod(jj, 2)
        blk = res.results[core]["out"][pair]  # [2, 128, CH, NX] f16
        arr = blk.transpose(1, 0, 2, 3).reshape(KMAX, YQ, NX)
        outs[bin_i][b, :, YQ * yq:YQ * (yq + 1), :] = arr[half * C:(half + 1) * C]
    return tuple(outs), res


def kernel(**inputs):
    out, _ = _run(inputs)
    return out


def kernel_traced(**inputs):
    """Like kernel() but also returns BassKernelResults (for test.py)."""
    return _run(inputs, trace=True)
